# revision 8
# baseline (speedup 1.0000x reference)
"""Causal self-attention (GQA + RoPE) Trainium2 Bass kernel.

Problem: B=2, T=2048, D=2048, H=16 q-heads, KVH=4 kv-heads, HD=128.
Sharding: 8 cores = batch(2) x kv-groups(4). Core (b, g) computes q-heads
4g..4g+3 and kv-head g for batch b, producing a partial output
y_g @ Wo[512g:512g+512, :]; the host sums the 4 partials per batch.

Device-side layout (per core):
  xT   = x[b].T                       [D, T]     (host-transposed)
  qkT  = wqk.T @ xT                   5x[128, T] (q/k transposed: head-dim on partitions)
  v    = x[b] @ Wv_g                  [T, 128]   (natural)
  RoPE via pair-swap matmul (PERM) + elementwise maps C/S (host-precomputed)
  S^T tiles = kT_tile.T @ qT          [tk=128, tq<=512]  -> exp (ACT, scaled) -> P^T
  y^T[:, sl] += v_tile.T @ P^T ;  l[1, sl] += ones.T @ P^T   (PSUM accumulate)
  y^T *= broadcast(1/l)  (PE ones-row broadcast + DVE mul)
  outp = sum_c yT_c.T @ Wo_g[c-block]            [T, D]  partial

All matmuls run as float32r (FP22 single-pass); matmul-feeding tiles are
declared float32r and every producer writes the float32r view (walrus
birverifier requires rounded producers). Causality is handled by narrowing
the diagonal tiles' matmuls to the valid column range plus a triangular
mask add on the 128-wide diagonal block.
"""

import sys
import numpy as np

if "/opt/trn_rl_repo" not in sys.path:
    sys.path.insert(0, "/opt/trn_rl_repo")

B, T, D = 2, 2048, 2048
H, KVH = 16, 4
HD = 128
P = 128
NKB = D // P            # 16 contraction blocks
NTK = T // P            # 16 key tiles
NSL = T // 512          # 4 query slices of 512
SCALE = float(1.0 / np.sqrt(HD))
NEG = -1.0e30

_CACHE = {}


def _build_nc():
    import concourse.mybir as mybir
    import concourse.tile as tile
    from concourse import bacc
    from contextlib import ExitStack

    F32 = mybir.dt.float32
    FR = mybir.dt.float32r
    Exp = mybir.ActivationFunctionType.Exp

    nc = bacc.Bacc("TRN2", target_bir_lowering=False, debug=False)

    xT_d = nc.dram_tensor("xT", [D, T], F32, kind="ExternalInput").ap()
    wqkv_d = nc.dram_tensor("wqkv", [D, 768], F32, kind="ExternalInput").ap()
    wo_d = nc.dram_tensor("wo", [512, D], F32, kind="ExternalInput").ap()
    ropeC_d = nc.dram_tensor("ropeC", [P, T], F32, kind="ExternalInput").ap()
    ropeS_d = nc.dram_tensor("ropeS", [P, T], F32, kind="ExternalInput").ap()
    perm_d = nc.dram_tensor("perm", [P, P], F32, kind="ExternalInput").ap()
    tri_d = nc.dram_tensor("tri", [P, P], F32, kind="ExternalInput").ap()
    onesc_d = nc.dram_tensor("onesc", [P, 1], F32, kind="ExternalInput").ap()
    onesr_d = nc.dram_tensor("onesr", [1, P], F32, kind="ExternalInput").ap()
    ident_d = nc.dram_tensor("ident", [P, P], F32, kind="ExternalInput").ap()
    outp_d = nc.dram_tensor("outp", [T, D], F32, kind="ExternalOutput").ap()

    with tile.TileContext(nc) as tc, ExitStack() as ctx:
        # ---- persistent pools -------------------------------------------
        singles = ctx.enter_context(tc.tile_pool(name="singles", bufs=1))
        qk_pool = ctx.enter_context(tc.tile_pool(name="qk", bufs=1))
        v_pool = ctx.enter_context(tc.tile_pool(name="vp", bufs=1))
        y_pool = ctx.enter_context(tc.tile_pool(name="yp", bufs=1))

        ropeC = singles.tile([P, T], F32)
        nc.sync.dma_start(out=ropeC, in_=ropeC_d)
        ropeS = singles.tile([P, T], F32)
        nc.sync.dma_start(out=ropeS, in_=ropeS_d)
        perm = singles.tile([P, P], FR)
        nc.sync.dma_start(out=perm, in_=perm_d.bitcast(FR))
        tri = singles.tile([P, P], F32)
        nc.sync.dma_start(out=tri, in_=tri_d)
        onesc = singles.tile([P, 1], FR)
        nc.sync.dma_start(out=onesc, in_=onesc_d.bitcast(FR))
        onesr = singles.tile([1, P], FR)
        nc.sync.dma_start(out=onesr, in_=onesr_d.bitcast(FR))
        ident = singles.tile([P, P], F32)
        nc.sync.dma_start(out=ident, in_=ident_d)

        # qkT[m]: m=0..3 q-heads, m=4 k-head; vT = v transposed [HD, T];
        # v_sb[:, tk, :] = v[128tk:128tk+128, :] natural (for PV lhsT)
        qkT = [qk_pool.tile([P, T], FR, name=f"qkT{m}") for m in range(5)]
        vT = qk_pool.tile([P, T], F32, name="vT")
        v_sb = v_pool.tile([P, NTK, HD], FR)
        yT = [y_pool.tile([P, T], FR, name=f"yT{h}") for h in range(4)]

        # ---- phase 1: projections ---------------------------------------
        with tc.tile_pool(name="wqkv", bufs=1) as wpool, \
             tc.tile_pool(name="xts", bufs=3) as xpool, \
             tc.tile_pool(name="qkps", bufs=1, space="PSUM") as qkps_pool:
            wqkv_sb = wpool.tile([P, NKB, 768], FR)
            nc.sync.dma_start(
                out=wqkv_sb, in_=wqkv_d.rearrange("(kb p) m -> p kb m", p=P).bitcast(FR))

            for n in range(NSL):
                qkps = [qkps_pool.tile([P, 512], F32, name=f"qkps{m}", tag=f"qkps{m}")
                        for m in range(6)]
                for kb in range(NKB):
                    xt = xpool.tile([P, 512], FR, name="xt", tag="xt")
                    nc.sync.dma_start(
                        out=xt,
                        in_=xT_d[kb * P:(kb + 1) * P, n * 512:(n + 1) * 512].bitcast(FR))
                    st, sp = (kb == 0), (kb == NKB - 1)
                    for m in range(6):
                        nc.tensor.matmul(
                            qkps[m], lhsT=wqkv_sb[:, kb, m * P:(m + 1) * P],
                            rhs=xt, start=st, stop=sp)
                for m in range(5):
                    nc.any.tensor_copy(out=qkT[m][:, n * 512:(n + 1) * 512], in_=qkps[m])
                nc.any.tensor_copy(out=vT[:, n * 512:(n + 1) * 512], in_=qkps[5])

        # ---- phase 1b: transpose vT -> v natural -------------------------
        with tc.tile_pool(name="vtps", bufs=2, space="PSUM") as vtpool:
            for tk in range(NTK):
                vtp = vtpool.tile([P, P], F32, name="vtp", tag="vtp")
                nc.tensor.transpose(vtp, in_=vT[:, tk * P:(tk + 1) * P], identity=ident)
                nc.any.tensor_copy(out=v_sb[:, tk, :], in_=vtp)

        # ---- phase 2: RoPE on qkT ---------------------------------------
        with tc.tile_pool(name="ropet", bufs=3) as rpool, \
             tc.tile_pool(name="swps", bufs=2, space="PSUM") as spool:
            for m in range(5):
                for n in range(NSL):
                    sl = slice(n * 512, (n + 1) * 512)
                    swp = spool.tile([P, 512], F32, name="swp", tag="swp")
                    nc.tensor.matmul(swp, lhsT=perm, rhs=qkT[m][:, sl],
                                     start=True, stop=True)
                    t1 = rpool.tile([P, 512], F32, name="t1", tag="t1")
                    nc.vector.tensor_mul(t1, qkT[m][:, sl].bitcast(F32), ropeC[:, sl])
                    t2 = rpool.tile([P, 512], F32, name="t2", tag="t2")
                    nc.vector.tensor_mul(t2, swp, ropeS[:, sl])
                    nc.vector.tensor_add(qkT[m][:, sl], t1, t2)

        # ---- phase 3: attention -----------------------------------------
        with tc.tile_pool(name="pts", bufs=4) as ptpool, \
             tc.tile_pool(name="lsb", bufs=2) as lpool, \
             tc.tile_pool(name="ytmp", bufs=2) as ytpool, \
             tc.tile_pool(name="stps", bufs=3, space="PSUM") as stpool, \
             tc.tile_pool(name="yps", bufs=2, space="PSUM") as ypool, \
             tc.tile_pool(name="lps", bufs=1, space="PSUM") as lppool, \
             tc.tile_pool(name="bcps", bufs=1, space="PSUM") as bcpool:
            kT = qkT[4]
            for h in range(4):
                qTh = qkT[h]
                for n in range(NSL):
                    sl = slice(n * 512, (n + 1) * 512)
                    ntk = 4 * n + 4
                    yps = ypool.tile([P, 512], F32, name="yps", tag="yps")
                    lps = lppool.tile([1, 512], F32, name="lps", tag="lps")
                    for tk in range(ntk):
                        j = tk - 4 * n
                        c0 = j * P if j >= 1 else 0    # first valid local column
                        stp = stpool.tile([P, 512], F32, name="stp", tag="stp")
                        nc.tensor.matmul(
                            stp[:, c0:], lhsT=kT[:, tk * P:(tk + 1) * P],
                            rhs=qTh[:, n * 512 + c0:(n + 1) * 512],
                            start=True, stop=True)
                        if j >= 0:
                            nc.vector.tensor_add(
                                stp[:, c0:c0 + P], stp[:, c0:c0 + P], tri)
                        pt = ptpool.tile([P, 512], FR, name="pt", tag="pt")
                        nc.scalar.activation(out=pt[:, c0:], in_=stp[:, c0:],
                                             func=Exp, scale=SCALE)
                        st, sp = (tk == 0), (tk == ntk - 1)
                        nc.tensor.matmul(yps[:, c0:], lhsT=v_sb[:, tk, :],
                                         rhs=pt[:, c0:], start=st, stop=sp)
                        nc.tensor.matmul(lps[:, c0:], lhsT=onesc,
                                         rhs=pt[:, c0:], start=st, stop=sp)
                    linv = lpool.tile([1, 512], FR, name="linv", tag="linv")
                    with nc.allow_low_precision(reason="softmax 1/l in fp32r"):
                        nc.vector.reciprocal(out=linv, in_=lps)
                    bcp = bcpool.tile([P, 512], F32, name="bcp", tag="bcp")
                    nc.tensor.matmul(bcp, lhsT=onesr, rhs=linv,
                                     start=True, stop=True)
                    ysb = ytpool.tile([P, 512], F32, name="ysb", tag="ysb")
                    nc.any.tensor_copy(out=ysb, in_=yps)
                    nc.vector.tensor_mul(yT[h][:, sl], ysb, bcp)

        # ---- phase 4: output projection ---------------------------------
        with tc.tile_pool(name="wos", bufs=1) as wopool, \
             tc.tile_pool(name="osb", bufs=2) as opool, \
             tc.tile_pool(name="ops", bufs=2, space="PSUM") as opsum:
            wo_sb = wopool.tile([P, 4, D], FR)
            nc.sync.dma_start(
                out=wo_sb, in_=wo_d.rearrange("(c p) d -> p c d", p=P).bitcast(FR))
            for t in range(NTK):
                outsb = opool.tile([P, D], F32, name="outsb", tag="outsb")
                for dsl in range(4):
                    ops = opsum.tile([P, 512], F32, name="ops", tag=f"ops{dsl}")
                    for c in range(4):
                        nc.tensor.matmul(
                            ops, lhsT=yT[c][:, t * P:(t + 1) * P],
                            rhs=wo_sb[:, c, dsl * 512:(dsl + 1) * 512],
                            start=(c == 0), stop=(c == 3))
                    nc.any.tensor_copy(out=outsb[:, dsl * 512:(dsl + 1) * 512], in_=ops)
                nc.sync.dma_start(out=outp_d[t * P:(t + 1) * P, :], in_=outsb)

    nc.compile()
    return nc


def _host_consts(freqs_cos, freqs_sin):
    C = np.repeat(np.asarray(freqs_cos, np.float32).T, 2, axis=0)
    S = np.repeat(np.asarray(freqs_sin, np.float32).T, 2, axis=0).copy()
    S[0::2] *= -1.0
    C = np.ascontiguousarray(C)
    S = np.ascontiguousarray(S)
    perm = np.zeros((P, P), np.float32)
    perm[np.arange(P), np.arange(P) ^ 1] = 1.0
    tri = np.where(np.arange(P)[:, None] <= np.arange(P)[None, :], 0.0, NEG).astype(np.float32)
    onesc = np.ones((P, 1), np.float32)
    onesr = np.ones((1, P), np.float32)
    return C, S, perm, tri, onesc, onesr


def _in_maps(x, freqs_cos, freqs_sin, Wq, Wk, Wv, Wo):
    C, S, perm, tri, onesc, onesr = _host_consts(freqs_cos, freqs_sin)
    xTb = [np.ascontiguousarray(np.asarray(x, np.float32)[b].T) for b in range(B)]
    Wq = np.asarray(Wq, np.float32); Wk = np.asarray(Wk, np.float32)
    Wv = np.asarray(Wv, np.float32); Wo = np.asarray(Wo, np.float32)
    ident = np.eye(P, dtype=np.float32)
    maps = []
    for b in range(B):
        for g in range(KVH):
            wqkv = np.ascontiguousarray(np.concatenate(
                [Wq[:, 512 * g:512 * (g + 1)], Wk[:, HD * g:HD * (g + 1)],
                 Wv[:, HD * g:HD * (g + 1)]], axis=1))
            maps.append({
                "xT": xTb[b],
                "wqkv": wqkv,
                "wo": np.ascontiguousarray(Wo[512 * g:512 * (g + 1), :]),
                "ropeC": C, "ropeS": S, "perm": perm, "tri": tri,
                "onesc": onesc, "onesr": onesr, "ident": ident,
            })
    return maps


def kernel(x, freqs_cos, freqs_sin, Wq, Wk, Wv, Wo):
    from concourse.bass_utils import run_bass_kernel_spmd

    if "nc" not in _CACHE:
        _CACHE["nc"] = _build_nc()
    nc = _CACHE["nc"]

    in_maps = _in_maps(x, freqs_cos, freqs_sin, Wq, Wk, Wv, Wo)
    res = run_bass_kernel_spmd(nc, in_maps, core_ids=list(range(8)))
    out = np.zeros((B, T, D), np.float32)
    for b in range(B):
        for g in range(KVH):
            out[b] += res.results[b * KVH + g]["outp"]
    return out
